# revision 72
# baseline (speedup 1.0000x reference)
"""Trainium2 Bass kernel for DUPN-style LSTM + windowed-softmax attention pooling.

Math (per batch element b):
  LSTM over T=128 steps (torch gate order), hidden H=512, input D=256.
  a[t] = sigmoid(x[t]·u1 + h[t]·u2), u1 = (v1@A1)^T, u2 = (v1@A2)^T  (folded)
  out[b,k,:] = softmax-pooled sum of h[t] over window t <= t_k, for 4 slots.

Sharding: data-parallel over batch, 32 per core x 8 cores, weights replicated.

v4 design - everything h-major (transposed):
  - Per-step tensors are [128, 128]: row p = within-chunk dim, col 32c+b
    (c = 128-dim chunk, b = batch). Same layout as an hsT slice, so the DVE
    op producing h writes hsT directly - no per-step PE transposes/copies.
  - z^T per gate g in one PSUM bank pzT [128, 512] (col 128g+32c+b), built
    from: 4 inject matmuls (identity stationary, bf16 xw ring moving,
    N=128) + 32 fp8 DoubleRow matmuls (W_hh chunks stationary [K=2x128,
    M=128], hsT8 slice moving [128,(2,32)]). Full-partition outputs: no
    tile_position, clean accumulation groups (one start/stop per bank).
  - xw^T = (x@W_ih^T)^T precomputed per 16-step macro-chunk: W_ih chunks
    stationary, x^T moving (N=512), PSUM -> bf16 ring with the bias folded
    in as a per-partition scalar (tensor_scalar). a1 = u1-stationary matmul
    over the same moving x^T -> a1s row. a2 = 4 tiny per-step matmuls
    (u2 chunk columns stationary, hsT slice moving) -> a2s row; both rows
    are assembled post-loop via PE transposes.
  - Recurrent weights and h in fp8e4 (DoubleRow); x path bf16; pooling
    reads the fp32r hsT, softmax/pooling in fp32.
"""
import sys

if "/opt/trn_rl_repo" not in sys.path:
    sys.path.insert(0, "/opt/trn_rl_repo")

import numpy as np
import ml_dtypes
import concourse.bass as bass
import concourse.bacc as bacc
import concourse.tile as tile
from concourse import mybir
from concourse.bass_utils import run_bass_kernel_spmd
from contextlib import ExitStack

F32 = mybir.dt.float32
F32R = mybir.dt.float32r
BF16 = mybir.dt.bfloat16
F8 = mybir.dt.float8e4
PM = mybir.MatmulPerfMode
AFT = mybir.ActivationFunctionType
ALU = mybir.AluOpType

T, BF, D, H, K, NC = 128, 256, 256, 512, 4, 8
BL = BF // NC          # 32 batch per core
G = 4 * H              # 2048
NEG_INF = -1e9
MS = 16                # steps per xw macro-chunk
NMC = T // MS          # 8 macro-chunks

# gate order in z/pzT columns: [f, i, g, o]
GF, GI, GG, GO = 0, 1, 2, 3

_cached = {}


def _build_program(t_steps=T):
    nc = bacc.Bacc()
    d_xT = nc.declare_dram_parameter("xT", [D, t_steps * BL], F32R, isOutput=False)
    d_wih = nc.declare_dram_parameter("wih", [D, G], F32R, isOutput=False)
    d_whh_dr = [nc.declare_dram_parameter(f"whh_dr{j}", [128, 2 * G], F8,
                                          isOutput=False) for j in range(2)]
    d_biasq = nc.declare_dram_parameter("biasq", [128, 16], F32, isOutput=False)
    d_u1c = nc.declare_dram_parameter("u1c", [128, 2], F32R, isOutput=False)
    d_u2c = nc.declare_dram_parameter("u2c", [128, 4], F8, isOutput=False)
    d_i128b = nc.declare_dram_parameter("i128b", [128, 128], BF16, isOutput=False)
    d_i32s = nc.declare_dram_parameter("i32s", [128, 32], F32, isOutput=False)
    d_i128 = nc.declare_dram_parameter("i128", [128, 128], F32, isOutput=False)
    d_maskneg = nc.declare_dram_parameter("maskneg", [BL, K * t_steps], F32, isOutput=False)
    d_valid = nc.declare_dram_parameter("valid", [BL, K], F32, isOutput=False)
    d_out = nc.declare_dram_parameter("out", [BL * K, H], F32, isOutput=True)

    with tile.TileContext(nc) as tc, ExitStack() as ctx:
        nv, ns, nt, ng = nc.vector, nc.scalar, nc.tensor, nc.gpsimd

        consts = ctx.enter_context(tc.tile_pool(name="consts", bufs=1))
        big = ctx.enter_context(tc.tile_pool(name="big", bufs=1))

        # ---- constants ----
        wih_sb = [consts.tile([128, G], F32R, tag=f"wih{i}", name=f"wih{i}")
                  for i in range(2)]
        for i in range(2):
            nc.sync.dma_start(wih_sb[i][:], d_wih[128 * i:128 * (i + 1), :])
        whh_sb = [consts.tile([128, 2 * G], F8, tag=f"whh{j}", name=f"whh{j}")
                  for j in range(2)]
        for j in range(2):
            nc.sync.dma_start(whh_sb[j][:], d_whh_dr[j][:])
        # [p, two, gate, cc, gd] view of the DoubleRow weights
        whh_v = [whh_sb[j][:].rearrange("p (two gate cc gd) -> p two gate cc gd",
                                        two=2, gate=4, cc=4) for j in range(2)]
        biasq_sb = consts.tile([128, 16], F32, tag="biasq")
        nc.sync.dma_start(biasq_sb[:], d_biasq[:])
        u1c_sb = consts.tile([128, 2], F32R, tag="u1c")
        nc.sync.dma_start(u1c_sb[:], d_u1c[:])
        u2c_sb = consts.tile([128, 4], F8, tag="u2c")
        nc.sync.dma_start(u2c_sb[:], d_u2c[:])
        i128b_sb = consts.tile([128, 128], BF16, tag="i128b")
        nc.sync.dma_start(i128b_sb[:], d_i128b[:])
        i32s_f = consts.tile([128, 32], F32, tag="i32s_f")
        nc.sync.dma_start(i32s_f[:], d_i32s[:])
        i128_r = consts.tile([128, 128], F32R, tag="i128_r")
        nc.sync.dma_start(i128_r[:], d_i128[:].bitcast(F32R))
        maskneg_sb = consts.tile([BL, K * t_steps], F32, tag="maskneg")
        nc.sync.dma_start(maskneg_sb[:], d_maskneg[:])
        valid_sb = consts.tile([BL, K], F32, tag="valid")
        nc.sync.dma_start(valid_sb[:], d_valid[:])

        # ---- persistent state ----
        hsT = big.tile([128, t_steps * 128], F32R, tag="hsT")      # [p, t*128+32c+b]
        hsT8 = big.tile([128, t_steps * 128], F8, tag="hsT8")      # fp8 shadow
        c_sb = big.tile([128, 128], F32, tag="c")                  # h-major
        a1s = big.tile([1, t_steps * BL], F32, tag="a1s")          # a1 row [1,(t,b)]
        a2s = big.tile([1, t_steps * BL], F32, tag="a2s")          # a2 row [1,(t,b)]

        # ---- loop pools ----
        loop_ctx = ExitStack()
        xt_pool = loop_ctx.enter_context(tc.tile_pool(name="xt", bufs=2))
        xw_pool = loop_ctx.enter_context(tc.tile_pool(name="xw", bufs=2))
        gate_pool = loop_ctx.enter_context(tc.tile_pool(name="gate", bufs=2))
        tmp_pool = loop_ctx.enter_context(tc.tile_pool(name="tmp", bufs=2))
        ps_xw = loop_ctx.enter_context(tc.tile_pool(name="ps_xw", bufs=2, space="PSUM"))
        ps_z = loop_ctx.enter_context(tc.tile_pool(name="ps_z", bufs=2, space="PSUM"))
        ps_a1 = loop_ctx.enter_context(tc.tile_pool(name="ps_a1", bufs=1, space="PSUM"))
        ps_a2 = loop_ctx.enter_context(tc.tile_pool(name="ps_a2", bufs=2, space="PSUM"))

        # xw macro-chunk: xwT ring [128, 16*512] bf16, cols (4g+cc)*512 + t16*32 + b
        xw_state = {}

        def emit_xw_piece(mc, piece):
            """xw macro-chunk mc in 10 PE pieces: 0 = DMA, 1..8 = 2 (g,cc)
            column blocks each (matmuls + biased ring copies), 9 = a1."""
            if piece == 0:
                xt = xt_pool.tile([128, 2 * 512], F32R, tag="xt", name=f"xt_{mc}")
                for kd in range(2):
                    nc.sync.dma_start(
                        xt[:, 512 * kd:512 * (kd + 1)],
                        d_xT[128 * kd:128 * (kd + 1), 512 * mc:512 * (mc + 1)])
                ring = xw_pool.tile([128, 16 * 512], BF16, tag="ring",
                                    name=f"ring_{mc}")
                xw_state[mc] = ring
                xw_state[(mc, "xt")] = xt
                return
            xt = xw_state[(mc, "xt")]
            ring = xw_state[mc]
            if piece <= 8:
                for gc in (2 * (piece - 1), 2 * (piece - 1) + 1):
                    pxw = ps_xw.tile([128, 512], F32, tag="pxw",
                                     name=f"pxw_{mc}_{gc}")
                    for kd in range(2):
                        nt.matmul(pxw[:], wih_sb[kd][:, 128 * gc:128 * (gc + 1)],
                                  xt[:, 512 * kd:512 * (kd + 1)],
                                  start=(kd == 0), stop=(kd == 1))
                    xw_state.setdefault("pending", []).append((mc, gc, pxw))
            else:
                pa1 = ps_a1.tile([1, 512], F32, tag="pa1", name=f"pa1_{mc}")
                for kd in range(2):
                    nt.matmul(pa1[:], u1c_sb[:, kd:kd + 1],
                              xt[:, 512 * kd:512 * (kd + 1)],
                              start=(kd == 0), stop=(kd == 1))
                ns.copy(a1s[:, 512 * mc:512 * (mc + 1)], pa1[:])
                xw_state.pop((mc - 2, "xt"), None)
                xw_state.pop(mc - 2, None)

        def drain_ring_copies(nmax):
            pend = xw_state.get("pending", [])
            for k_ in range(min(nmax, len(pend))):
                mc, gc, pxw = pend.pop(0)
                ring = xw_state[mc]
                if k_ % 2 == 0:
                    nv.tensor_scalar(out=ring[:, 512 * gc:512 * (gc + 1)],
                                     in0=pxw[:], scalar1=biasq_sb[:, gc:gc + 1],
                                     scalar2=0.0, op0=ALU.add, op1=ALU.bypass)
                else:
                    ns.add(ring[:, 512 * gc:512 * (gc + 1)], pxw[:],
                           biasq_sb[:, gc:gc + 1])

        def emit_inject(pz, t, last):
            """xw ring cols for step t -> 4 per-gate injections starting z^T psum."""
            t16 = t % MS
            ring = xw_state[t // MS]
            rv = ring[:].rearrange("p (gc s b) -> p gc s b", gc=16, s=MS)
            nt.matmul(pz[:], i128b_sb[:], rv[:, :, t16, :], start=True, stop=last)

        for piece in range(10):
            emit_xw_piece(0, piece)
        drain_ring_copies(99)
        pz = ps_z.tile([128, 512], F32, tag="pz", name="pz0")
        emit_inject(pz, 0, last=True)

        CFI = slice(0, 256)
        CF, CI, CG, CO = (slice(128 * g_, 128 * (g_ + 1)) for g_ in range(4))

        for t in range(t_steps):
            # xw prefetch piece for the next macro (PE filler before k matmuls)
            t16p = t % MS
            if t16p < 10 and t // MS + 1 < NMC:
                emit_xw_piece(t // MS + 1, t16p)
            # ---- recurrent matmuls (t>0): fp8 DR, W stationary, hsT8 moving ----
            if t > 0:
                hb = (t - 1) * 128
                mv = [hsT8[:, hb + 64 * j:hb + 64 * (j + 1)].rearrange(
                    "p (two b) -> p two b", two=2) for j in range(2)]
                # f,i finish after 16 MMs (their sigmoid heads the chain)
                for j, g_ in ((0, GF), (0, GI), (1, GF), (1, GI),
                              (0, GG), (0, GO), (1, GG), (1, GO)):
                    for cc in range(4):
                        nt.matmul(pz[:, 128 * g_ + 32 * cc:
                                     128 * g_ + 32 * (cc + 1)],
                                  whh_v[j][:, :, g_, cc, :], mv[j],
                                  start=False,
                                  stop=(j == 1 and g_ == GO and cc == 3),
                                  perf_mode=PM.DoubleRow)
            # ---- gates ----
            sg_fi = gate_pool.tile([128, 256], F32, tag="sgfi")
            ns.activation(sg_fi[:], pz[:, CFI], AFT.Sigmoid)
            gg = gate_pool.tile([128, 128], F32, tag="gg")
            ns.activation(gg[:], pz[:, CG], AFT.Tanh)
            sg_o = gate_pool.tile([128, 128], F32, tag="sgo")
            ns.activation(sg_o[:], pz[:, CO], AFT.Sigmoid)
            # ---- c update ----
            if t == 0:
                nv.tensor_tensor(c_sb[:], sg_fi[:, 128:256], gg[:], op=ALU.mult)
            else:
                tfc = tmp_pool.tile([128, 128], F32, tag="tfc")
                nv.tensor_tensor(tfc[:], sg_fi[:, 0:128], c_sb[:], op=ALU.mult)
                tig = tmp_pool.tile([128, 128], F32, tag="tig")
                nv.tensor_tensor(tig[:], sg_fi[:, 128:256], gg[:], op=ALU.mult)
                nv.tensor_tensor(c_sb[:], tfc[:], tig[:], op=ALU.add)
            tcs = tmp_pool.tile([128, 128], F32, tag="tcs")
            ns.activation(tcs[:], c_sb[:], AFT.Tanh)
            # fp8 h first (gates next step's matmuls), f32r pooling copy after
            nv.tensor_tensor(hsT8[:, t * 128:(t + 1) * 128], sg_o[:], tcs[:],
                             op=ALU.mult)
            nv.tensor_tensor(hsT[:, t * 128:(t + 1) * 128], sg_o[:], tcs[:],
                             op=ALU.mult)
            # ---- a2[t] = h·u2: fp8 matmuls on hsT8 (ready before the f32r h);
            # u2 is host-scaled x16 into fp8 range, undone in the copy ----
            pa2 = ps_a2.tile([1, 32], F32, tag="pa2", name=f"pa2_{t}")
            for cq in range(4):
                nt.matmul(pa2[:], u2c_sb[:, cq:cq + 1],
                          hsT8[:, t * 128 + 32 * cq:t * 128 + 32 * (cq + 1)],
                          start=(cq == 0), stop=(cq == 3))
            nv.tensor_scalar(out=a2s[:, t * BL:(t + 1) * BL], in0=pa2[:],
                             scalar1=1.0 / 16.0, scalar2=0.0,
                             op0=ALU.mult, op1=ALU.bypass)
            # ---- PE: inject t+1 ----
            if t + 1 < t_steps:
                pz = ps_z.tile([128, 512], F32, tag="pz", name=f"pz{t + 1}")
                emit_inject(pz, t + 1, last=False)
            drain_ring_copies(2)

        loop_ctx.close()

        # ---- post-loop ----
        post = ctx.enter_context(tc.tile_pool(name="post", bufs=1))
        ps_t = ctx.enter_context(tc.tile_pool(name="ps_t", bufs=3, space="PSUM"))
        ps_sm = ctx.enter_context(tc.tile_pool(name="ps_sm", bufs=1, space="PSUM"))
        ps_pool = ctx.enter_context(tc.tile_pool(name="ps_pool", bufs=2, space="PSUM"))
        stg_pool = ctx.enter_context(tc.tile_pool(name="stg", bufs=4))
        hsb_pool = ctx.enter_context(tc.tile_pool(name="hsb", bufs=2))

        # assemble abp[b, t] from a1s+a2s rows [1, (t,b)]:
        # asum[0, t*32+b]; transpose 128-col chunks -> [128=(t4,b), 32 chunks]
        asum = post.tile([1, t_steps * BL], F32, tag="asum")
        nv.tensor_tensor(asum[:], a1s[:], a2s[:], op=ALU.add)
        pat = ps_sm.tile([128, 32], F32, tag="pat")
        for ch in range(32):
            nt.transpose(pat[:, ch:ch + 1],
                         asum[:, 128 * ch:128 * (ch + 1)], i32s_f[0:1, 0:1])
        ach = post.tile([128, 32], F32, tag="ach")
        ns.copy(ach[:], pat[:])
        # abp[b, 4r+c'] = ach[32c'+b, r]  (same trick as old a1 assembly)
        abp = post.tile([BL, t_steps], F32, tag="abp")
        for c in range(4):
            nv.tensor_copy(abp[:].rearrange("b (r c) -> b r c", c=4)[:, :, c],
                           ach[32 * c:32 * (c + 1), :])
        ns.activation(abp[:], abp[:], AFT.Sigmoid)

        # softmax per slot k -> wT [t, 4b+k] (fp32r for the pooling matmul)
        wT = post.tile([t_steps, K * BL], F32R, tag="wT")
        for k in range(K):
            sc = post.tile([BL, t_steps], F32, tag=f"sc{k}")
            nv.tensor_tensor(sc[:], abp[:],
                             maskneg_sb[:, t_steps * k:t_steps * (k + 1)], op=ALU.add)
            mneg = post.tile([BL, 1], F32, tag=f"mneg{k}")
            nv.tensor_reduce(mneg[:], sc[:], axis=mybir.AxisListType.X,
                             op=ALU.max, negate=True)
            ek = post.tile([BL, t_steps], F32, tag=f"ek{k}")
            sk = post.tile([BL, 1], F32, tag=f"sk{k}")
            ns.activation(ek[:], sc[:], AFT.Exp, bias=mneg[:], accum_out=sk[:])
            rk = post.tile([BL, 1], F32, tag=f"rk{k}")
            nv.reciprocal(rk[:], sk[:])
            wk = post.tile([BL, t_steps], F32, tag=f"wk{k}")
            nv.tensor_scalar(out=wk[:], in0=ek[:], scalar1=rk[:],
                             scalar2=valid_sb[:, k:k + 1], op0=ALU.mult, op1=ALU.mult)
            pwT = ps_sm.tile([128, 32], F32, tag="pwT")
            nt.transpose(pwT[0:t_steps, :], wk[:], i32s_f[0:32, :])
            nv.tensor_copy(wT[:].rearrange("t (b k) -> t b k", k=4)[:, :, k],
                           pwT[0:t_steps, :])

        # pooling: per b, rebuild hs_b [t, h] via PE transposes, then [4,T]@[T,H]
        hsT_r = hsT[:].rearrange("p (t c b) -> p t c b", c=4, b=BL)
        for b in range(BL):
            hsb = hsb_pool.tile([t_steps, H], F32R, tag="hsb")
            for half in range(2):
                pt = ps_t.tile([128, 256], F32R, tag="pt")
                for cc in range(2):
                    c = 2 * half + cc
                    nt.transpose(pt[0:t_steps, 128 * cc:128 * (cc + 1)],
                                 hsT_r[:, :, c, b], i128_r[:])
                if half == 0:
                    ns.copy(hsb[:, 0:256], pt[0:t_steps, :])
                else:
                    nv.tensor_copy(hsb[:, 256:512], pt[0:t_steps, :])
            pp = ps_pool.tile([K, H], F32, tag="pp")
            nt.matmul(pp[:], wT[0:t_steps, 4 * b:4 * (b + 1)], hsb[:],
                      start=True, stop=True)
            so = stg_pool.tile([K, H], F32, tag="so")
            if b % 2 == 0:
                ns.copy(so[:], pp[:])
            else:
                nv.tensor_copy(so[:], pp[:])
            nc.sync.dma_start(d_out[K * b:K * (b + 1), :], so[:])

    nc.compile()
    return nc


def _host_prep(x, W_ih, W_hh, b_ih, b_hh, A1, A2, v1, lengths, label_len):
    assert int(label_len) == K
    # z gate columns: [f, i, g, o]
    perm = np.concatenate([np.arange(512, 1024), np.arange(0, 512),
                           np.arange(1024, 1536), np.arange(1536, 2048)])
    wih = np.ascontiguousarray(W_ih[perm].T, dtype=np.float32)          # [256, 2048]
    whh = np.ascontiguousarray(W_hh[perm].T, dtype=np.float32)          # [512, 2048]
    whh4 = whh.reshape(4, 128, G)
    whh_dr = [np.ascontiguousarray(
        np.concatenate([whh4[2 * j], whh4[2 * j + 1]], axis=1)
    ).astype(ml_dtypes.float8_e4m3) for j in range(2)]                  # [128, 2G] fp8
    bias = ((b_ih + b_hh)[perm]).astype(np.float32)                     # [2048]
    biasq = np.ascontiguousarray(bias.reshape(16, 128).T)               # [128, 16]
    u1 = (v1 @ A1)[0].astype(np.float32)                                # [256]
    u2 = (v1 @ A2)[0].astype(np.float32)                                # [512]
    u1c = np.ascontiguousarray(u1.reshape(2, 128).T)                    # [128, 2]
    u2c = np.ascontiguousarray(u2.reshape(4, 128).T * 16.0).astype(
        ml_dtypes.float8_e4m3)                                          # [128, 4] fp8

    i32s = np.zeros((128, 32), dtype=np.float32)
    i32s[np.arange(128), np.arange(128) % 32] = 1.0
    i128 = np.eye(128, dtype=np.float32)
    i128b = i128.astype(ml_dtypes.bfloat16)

    shared = dict(wih=wih, whh_dr0=whh_dr[0], whh_dr1=whh_dr[1], biasq=biasq,
                  u1c=u1c, u2c=u2c, i32s=i32s, i128=i128, i128b=i128b)

    in_maps = []
    for cidx in range(NC):
        sl = slice(cidx * BL, (cidx + 1) * BL)
        xc = x[:, sl, :]                                                # [T, 32, D]
        xT = np.ascontiguousarray(xc.reshape(T * BL, D).T, dtype=np.float32)
        ln = lengths[sl].astype(np.int64)
        t_start = np.maximum(ln - K, 0)
        t_k = t_start[:, None] + np.arange(K)[None, :]                  # [32, 4]
        valid = (t_k <= (ln[:, None] - 1))                              # [32, 4]
        tt = np.arange(T)
        mask = (tt[None, None, :] <= t_k[:, :, None]) & valid[:, :, None]  # [b, k, t]
        maskneg = np.where(mask, 0.0, NEG_INF).astype(np.float32)
        maskneg = np.ascontiguousarray(maskneg.reshape(BL, K * T))      # k-major cols
        in_maps.append(dict(shared, xT=xT, maskneg=maskneg,
                            valid=valid.astype(np.float32)))
    return in_maps


def kernel(**inputs) -> np.ndarray:
    inputs = {k: np.asarray(v) if not np.isscalar(v) else v for k, v in inputs.items()}
    in_maps = _host_prep(**inputs)
    if "nc" not in _cached:
        _cached["nc"] = _build_program()
    nc = _cached["nc"]
    res = run_bass_kernel_spmd(nc, in_maps, core_ids=list(range(NC)))
    outs = []
    for cidx in range(NC):
        o = res.results[cidx]["out"]                                    # [128, 512]
        outs.append(o.reshape(BL, K, H))
    return np.concatenate(outs, axis=0).astype(np.float32)              # [256, 4, 512]


# revision 73
# speedup vs baseline: 1.0162x; 1.0162x over previous
"""Trainium2 Bass kernel for DUPN-style LSTM + windowed-softmax attention pooling.

Math (per batch element b):
  LSTM over T=128 steps (torch gate order), hidden H=512, input D=256.
  a[t] = sigmoid(x[t]·u1 + h[t]·u2), u1 = (v1@A1)^T, u2 = (v1@A2)^T  (folded)
  out[b,k,:] = softmax-pooled sum of h[t] over window t <= t_k, for 4 slots.

Sharding: data-parallel over batch, 32 per core x 8 cores, weights replicated.

v4 design - everything h-major (transposed):
  - Per-step tensors are [128, 128]: row p = within-chunk dim, col 32c+b
    (c = 128-dim chunk, b = batch). Same layout as an hsT slice, so the DVE
    op producing h writes hsT directly - no per-step PE transposes/copies.
  - z^T per gate g in one PSUM bank pzT [128, 512] (col 128g+32c+b), built
    from: 4 inject matmuls (identity stationary, bf16 xw ring moving,
    N=128) + 32 fp8 DoubleRow matmuls (W_hh chunks stationary [K=2x128,
    M=128], hsT8 slice moving [128,(2,32)]). Full-partition outputs: no
    tile_position, clean accumulation groups (one start/stop per bank).
  - xw^T = (x@W_ih^T)^T precomputed per 16-step macro-chunk: W_ih chunks
    stationary, x^T moving (N=512), PSUM -> bf16 ring with the bias folded
    in as a per-partition scalar (tensor_scalar). a1 = u1-stationary matmul
    over the same moving x^T -> a1s row. a2 = 4 tiny per-step matmuls
    (u2 chunk columns stationary, hsT slice moving) -> a2s row; both rows
    are assembled post-loop via PE transposes.
  - Recurrent weights and h in fp8e4 (DoubleRow); x path bf16; pooling
    reads the fp32r hsT, softmax/pooling in fp32.
"""
import sys

if "/opt/trn_rl_repo" not in sys.path:
    sys.path.insert(0, "/opt/trn_rl_repo")

import numpy as np
import ml_dtypes
import concourse.bass as bass
import concourse.bacc as bacc
import concourse.tile as tile
from concourse import mybir
from concourse.bass_utils import run_bass_kernel_spmd
from contextlib import ExitStack

F32 = mybir.dt.float32
F32R = mybir.dt.float32r
BF16 = mybir.dt.bfloat16
F8 = mybir.dt.float8e4
PM = mybir.MatmulPerfMode
AFT = mybir.ActivationFunctionType
ALU = mybir.AluOpType

T, BF, D, H, K, NC = 128, 256, 256, 512, 4, 8
BL = BF // NC          # 32 batch per core
G = 4 * H              # 2048
NEG_INF = -1e9
MS = 16                # steps per xw macro-chunk
NMC = T // MS          # 8 macro-chunks

# gate order in z/pzT columns: [f, i, g, o]
GF, GI, GG, GO = 0, 1, 2, 3

_cached = {}


def _build_program(t_steps=T):
    nc = bacc.Bacc()
    d_xT = nc.declare_dram_parameter("xT", [D, t_steps * BL], F32R, isOutput=False)
    d_wih = nc.declare_dram_parameter("wih", [D, G], F32R, isOutput=False)
    d_whh_dr = [nc.declare_dram_parameter(f"whh_dr{j}", [128, 2 * G], F8,
                                          isOutput=False) for j in range(2)]
    d_biasq = nc.declare_dram_parameter("biasq", [128, 16], F32, isOutput=False)
    d_u1c = nc.declare_dram_parameter("u1c", [128, 2], F32R, isOutput=False)
    d_u2c = nc.declare_dram_parameter("u2c", [128, 4], F8, isOutput=False)
    d_i128b = nc.declare_dram_parameter("i128b", [128, 128], BF16, isOutput=False)
    d_i32s = nc.declare_dram_parameter("i32s", [128, 32], F32, isOutput=False)
    d_i128 = nc.declare_dram_parameter("i128", [128, 128], F32, isOutput=False)
    d_maskneg = nc.declare_dram_parameter("maskneg", [BL, K * t_steps], F32, isOutput=False)
    d_valid = nc.declare_dram_parameter("valid", [BL, K], F32, isOutput=False)
    d_out = nc.declare_dram_parameter("out", [BL * K, H], F32, isOutput=True)

    with tile.TileContext(nc) as tc, ExitStack() as ctx:
        nv, ns, nt, ng = nc.vector, nc.scalar, nc.tensor, nc.gpsimd

        consts = ctx.enter_context(tc.tile_pool(name="consts", bufs=1))
        big = ctx.enter_context(tc.tile_pool(name="big", bufs=1))

        # ---- constants ----
        wih_sb = [consts.tile([128, G], F32R, tag=f"wih{i}", name=f"wih{i}")
                  for i in range(2)]
        for i in range(2):
            nc.sync.dma_start(wih_sb[i][:], d_wih[128 * i:128 * (i + 1), :])
        whh_sb = [consts.tile([128, 2 * G], F8, tag=f"whh{j}", name=f"whh{j}")
                  for j in range(2)]
        # whh DMAs deferred until after the macro-0 x DMA (emit_whh_dmas)
        def emit_whh_dmas():
            for j in range(2):
                nc.sync.dma_start(whh_sb[j][:], d_whh_dr[j][:])
        # [p, two, gate, cc, gd] view of the DoubleRow weights
        whh_v = [whh_sb[j][:].rearrange("p (two gate cc gd) -> p two gate cc gd",
                                        two=2, gate=4, cc=4) for j in range(2)]
        biasq_sb = consts.tile([128, 16], F32, tag="biasq")
        nc.sync.dma_start(biasq_sb[:], d_biasq[:])
        u1c_sb = consts.tile([128, 2], F32R, tag="u1c")
        nc.sync.dma_start(u1c_sb[:], d_u1c[:])
        u2c_sb = consts.tile([128, 4], F8, tag="u2c")
        nc.sync.dma_start(u2c_sb[:], d_u2c[:])
        i128b_sb = consts.tile([128, 128], BF16, tag="i128b")
        nc.sync.dma_start(i128b_sb[:], d_i128b[:])
        # post-loop-only constants: tiles here, DMAs deferred into the loop
        i32s_f = consts.tile([128, 32], F32, tag="i32s_f")
        i128_r = consts.tile([128, 128], F32R, tag="i128_r")
        maskneg_sb = consts.tile([BL, K * t_steps], F32, tag="maskneg")
        valid_sb = consts.tile([BL, K], F32, tag="valid")

        def emit_post_const_dmas():
            nc.sync.dma_start(i32s_f[:], d_i32s[:])
            nc.sync.dma_start(i128_r[:], d_i128[:].bitcast(F32R))
            nc.sync.dma_start(maskneg_sb[:], d_maskneg[:])
            nc.sync.dma_start(valid_sb[:], d_valid[:])

        # ---- persistent state ----
        hsT = big.tile([128, t_steps * 128], F32R, tag="hsT")      # [p, t*128+32c+b]
        hsT8 = big.tile([128, t_steps * 128], F8, tag="hsT8")      # fp8 shadow
        c_sb = big.tile([128, 128], F32, tag="c")                  # h-major
        a1s = big.tile([1, t_steps * BL], F32, tag="a1s")          # a1 row [1,(t,b)]
        a2s = big.tile([1, t_steps * BL], F32, tag="a2s")          # a2 row [1,(t,b)]

        # ---- loop pools ----
        loop_ctx = ExitStack()
        xt_pool = loop_ctx.enter_context(tc.tile_pool(name="xt", bufs=2))
        xw_pool = loop_ctx.enter_context(tc.tile_pool(name="xw", bufs=2))
        gate_pool = loop_ctx.enter_context(tc.tile_pool(name="gate", bufs=2))
        tmp_pool = loop_ctx.enter_context(tc.tile_pool(name="tmp", bufs=2))
        ps_xw = loop_ctx.enter_context(tc.tile_pool(name="ps_xw", bufs=2, space="PSUM"))
        ps_z = loop_ctx.enter_context(tc.tile_pool(name="ps_z", bufs=2, space="PSUM"))
        ps_a1 = loop_ctx.enter_context(tc.tile_pool(name="ps_a1", bufs=1, space="PSUM"))
        ps_a2 = loop_ctx.enter_context(tc.tile_pool(name="ps_a2", bufs=2, space="PSUM"))

        # xw macro-chunk: xwT ring [128, 16*512] bf16, cols (4g+cc)*512 + t16*32 + b
        xw_state = {}

        def emit_xw_piece(mc, piece):
            """xw macro-chunk mc in 10 PE pieces: 0 = DMA, 1..8 = 2 (g,cc)
            column blocks each (matmuls + biased ring copies), 9 = a1."""
            if piece == 0:
                xt = xt_pool.tile([128, 2 * 512], F32R, tag="xt", name=f"xt_{mc}")
                for kd in range(2):
                    nc.sync.dma_start(
                        xt[:, 512 * kd:512 * (kd + 1)],
                        d_xT[128 * kd:128 * (kd + 1), 512 * mc:512 * (mc + 1)])
                ring = xw_pool.tile([128, 16 * 512], BF16, tag="ring",
                                    name=f"ring_{mc}")
                xw_state[mc] = ring
                xw_state[(mc, "xt")] = xt
                return
            xt = xw_state[(mc, "xt")]
            ring = xw_state[mc]
            if piece <= 8:
                for gc in (2 * (piece - 1), 2 * (piece - 1) + 1):
                    pxw = ps_xw.tile([128, 512], F32, tag="pxw",
                                     name=f"pxw_{mc}_{gc}")
                    for kd in range(2):
                        nt.matmul(pxw[:], wih_sb[kd][:, 128 * gc:128 * (gc + 1)],
                                  xt[:, 512 * kd:512 * (kd + 1)],
                                  start=(kd == 0), stop=(kd == 1))
                    xw_state.setdefault("pending", []).append((mc, gc, pxw))
            else:
                pa1 = ps_a1.tile([1, 512], F32, tag="pa1", name=f"pa1_{mc}")
                for kd in range(2):
                    nt.matmul(pa1[:], u1c_sb[:, kd:kd + 1],
                              xt[:, 512 * kd:512 * (kd + 1)],
                              start=(kd == 0), stop=(kd == 1))
                ns.copy(a1s[:, 512 * mc:512 * (mc + 1)], pa1[:])
                xw_state.pop((mc - 2, "xt"), None)
                xw_state.pop(mc - 2, None)

        def drain_ring_copies(nmax):
            pend = xw_state.get("pending", [])
            for k_ in range(min(nmax, len(pend))):
                mc, gc, pxw = pend.pop(0)
                ring = xw_state[mc]
                if k_ % 2 == 0:
                    nv.tensor_scalar(out=ring[:, 512 * gc:512 * (gc + 1)],
                                     in0=pxw[:], scalar1=biasq_sb[:, gc:gc + 1],
                                     scalar2=0.0, op0=ALU.add, op1=ALU.bypass)
                else:
                    ns.add(ring[:, 512 * gc:512 * (gc + 1)], pxw[:],
                           biasq_sb[:, gc:gc + 1])

        def emit_inject(pz, t, last):
            """xw ring cols for step t -> 4 per-gate injections starting z^T psum."""
            t16 = t % MS
            ring = xw_state[t // MS]
            rv = ring[:].rearrange("p (gc s b) -> p gc s b", gc=16, s=MS)
            nt.matmul(pz[:], i128b_sb[:], rv[:, :, t16, :], start=True, stop=last)

        emit_xw_piece(0, 0)          # x DMA for macro 0 ahead of the weights
        emit_whh_dmas()
        for piece in range(1, 10):
            emit_xw_piece(0, piece)
        drain_ring_copies(99)
        pz = ps_z.tile([128, 512], F32, tag="pz", name="pz0")
        emit_inject(pz, 0, last=True)

        CFI = slice(0, 256)
        CF, CI, CG, CO = (slice(128 * g_, 128 * (g_ + 1)) for g_ in range(4))

        for t in range(t_steps):
            if t == 2:
                emit_post_const_dmas()
            # xw prefetch piece for the next macro (PE filler before k matmuls)
            t16p = t % MS
            if t16p < 10 and t // MS + 1 < NMC:
                emit_xw_piece(t // MS + 1, t16p)
            # ---- recurrent matmuls (t>0): fp8 DR, W stationary, hsT8 moving ----
            if t > 0:
                hb = (t - 1) * 128
                mv = [hsT8[:, hb + 64 * j:hb + 64 * (j + 1)].rearrange(
                    "p (two b) -> p two b", two=2) for j in range(2)]
                # f,i finish after 16 MMs (their sigmoid heads the chain)
                for j, g_ in ((0, GF), (0, GI), (1, GF), (1, GI),
                              (0, GG), (0, GO), (1, GG), (1, GO)):
                    for cc in range(4):
                        nt.matmul(pz[:, 128 * g_ + 32 * cc:
                                     128 * g_ + 32 * (cc + 1)],
                                  whh_v[j][:, :, g_, cc, :], mv[j],
                                  start=False,
                                  stop=(j == 1 and g_ == GO and cc == 3),
                                  perf_mode=PM.DoubleRow)
            # ---- gates ----
            sg_fi = gate_pool.tile([128, 256], F32, tag="sgfi")
            ns.activation(sg_fi[:], pz[:, CFI], AFT.Sigmoid)
            gg = gate_pool.tile([128, 128], F32, tag="gg")
            ns.activation(gg[:], pz[:, CG], AFT.Tanh)
            sg_o = gate_pool.tile([128, 128], F32, tag="sgo")
            ns.activation(sg_o[:], pz[:, CO], AFT.Sigmoid)
            # ---- c update ----
            if t == 0:
                nv.tensor_tensor(c_sb[:], sg_fi[:, 128:256], gg[:], op=ALU.mult)
            else:
                tfc = tmp_pool.tile([128, 128], F32, tag="tfc")
                nv.tensor_tensor(tfc[:], sg_fi[:, 0:128], c_sb[:], op=ALU.mult)
                tig = tmp_pool.tile([128, 128], F32, tag="tig")
                nv.tensor_tensor(tig[:], sg_fi[:, 128:256], gg[:], op=ALU.mult)
                nv.tensor_tensor(c_sb[:], tfc[:], tig[:], op=ALU.add)
            tcs = tmp_pool.tile([128, 128], F32, tag="tcs")
            ns.activation(tcs[:], c_sb[:], AFT.Tanh)
            # fp8 h first (gates next step's matmuls), f32r pooling copy after
            nv.tensor_tensor(hsT8[:, t * 128:(t + 1) * 128], sg_o[:], tcs[:],
                             op=ALU.mult)
            nv.tensor_tensor(hsT[:, t * 128:(t + 1) * 128], sg_o[:], tcs[:],
                             op=ALU.mult)
            # ---- a2[t] = h·u2: fp8 matmuls on hsT8 (ready before the f32r h);
            # u2 is host-scaled x16 into fp8 range, undone in the copy ----
            pa2 = ps_a2.tile([1, 32], F32, tag="pa2", name=f"pa2_{t}")
            for cq in range(4):
                nt.matmul(pa2[:], u2c_sb[:, cq:cq + 1],
                          hsT8[:, t * 128 + 32 * cq:t * 128 + 32 * (cq + 1)],
                          start=(cq == 0), stop=(cq == 3))
            nv.tensor_scalar(out=a2s[:, t * BL:(t + 1) * BL], in0=pa2[:],
                             scalar1=1.0 / 16.0, scalar2=0.0,
                             op0=ALU.mult, op1=ALU.bypass)
            # ---- PE: inject t+1 ----
            if t + 1 < t_steps:
                pz = ps_z.tile([128, 512], F32, tag="pz", name=f"pz{t + 1}")
                emit_inject(pz, t + 1, last=False)
            drain_ring_copies(2)

        loop_ctx.close()

        # ---- post-loop ----
        post = ctx.enter_context(tc.tile_pool(name="post", bufs=1))
        ps_t = ctx.enter_context(tc.tile_pool(name="ps_t", bufs=3, space="PSUM"))
        ps_sm = ctx.enter_context(tc.tile_pool(name="ps_sm", bufs=1, space="PSUM"))
        ps_pool = ctx.enter_context(tc.tile_pool(name="ps_pool", bufs=2, space="PSUM"))
        stg_pool = ctx.enter_context(tc.tile_pool(name="stg", bufs=4))
        hsb_pool = ctx.enter_context(tc.tile_pool(name="hsb", bufs=2))

        # assemble abp[b, t] from a1s+a2s rows [1, (t,b)]:
        # asum[0, t*32+b]; transpose 128-col chunks -> [128=(t4,b), 32 chunks]
        asum = post.tile([1, t_steps * BL], F32, tag="asum")
        nv.tensor_tensor(asum[:], a1s[:], a2s[:], op=ALU.add)
        pat = ps_sm.tile([128, 32], F32, tag="pat")
        for ch in range(32):
            nt.transpose(pat[:, ch:ch + 1],
                         asum[:, 128 * ch:128 * (ch + 1)], i32s_f[0:1, 0:1])
        ach = post.tile([128, 32], F32, tag="ach")
        ns.copy(ach[:], pat[:])
        # abp[b, 4r+c'] = ach[32c'+b, r]  (same trick as old a1 assembly)
        abp = post.tile([BL, t_steps], F32, tag="abp")
        for c in range(4):
            nv.tensor_copy(abp[:].rearrange("b (r c) -> b r c", c=4)[:, :, c],
                           ach[32 * c:32 * (c + 1), :])
        ns.activation(abp[:], abp[:], AFT.Sigmoid)

        # softmax per slot k -> wT [t, 4b+k] (fp32r for the pooling matmul)
        wT = post.tile([t_steps, K * BL], F32R, tag="wT")
        for k in range(K):
            sc = post.tile([BL, t_steps], F32, tag=f"sc{k}")
            nv.tensor_tensor(sc[:], abp[:],
                             maskneg_sb[:, t_steps * k:t_steps * (k + 1)], op=ALU.add)
            mneg = post.tile([BL, 1], F32, tag=f"mneg{k}")
            nv.tensor_reduce(mneg[:], sc[:], axis=mybir.AxisListType.X,
                             op=ALU.max, negate=True)
            ek = post.tile([BL, t_steps], F32, tag=f"ek{k}")
            sk = post.tile([BL, 1], F32, tag=f"sk{k}")
            ns.activation(ek[:], sc[:], AFT.Exp, bias=mneg[:], accum_out=sk[:])
            rk = post.tile([BL, 1], F32, tag=f"rk{k}")
            nv.reciprocal(rk[:], sk[:])
            wk = post.tile([BL, t_steps], F32, tag=f"wk{k}")
            nv.tensor_scalar(out=wk[:], in0=ek[:], scalar1=rk[:],
                             scalar2=valid_sb[:, k:k + 1], op0=ALU.mult, op1=ALU.mult)
            pwT = ps_sm.tile([128, 32], F32, tag="pwT")
            nt.transpose(pwT[0:t_steps, :], wk[:], i32s_f[0:32, :])
            nv.tensor_copy(wT[:].rearrange("t (b k) -> t b k", k=4)[:, :, k],
                           pwT[0:t_steps, :])

        # pooling: per b, rebuild hs_b [t, h] via PE transposes, then [4,T]@[T,H]
        hsT_r = hsT[:].rearrange("p (t c b) -> p t c b", c=4, b=BL)
        for b in range(BL):
            hsb = hsb_pool.tile([t_steps, H], F32R, tag="hsb")
            for half in range(2):
                pt = ps_t.tile([128, 256], F32R, tag="pt")
                for cc in range(2):
                    c = 2 * half + cc
                    nt.transpose(pt[0:t_steps, 128 * cc:128 * (cc + 1)],
                                 hsT_r[:, :, c, b], i128_r[:])
                if half == 0:
                    ns.copy(hsb[:, 0:256], pt[0:t_steps, :])
                else:
                    nv.tensor_copy(hsb[:, 256:512], pt[0:t_steps, :])
            pp = ps_pool.tile([K, H], F32, tag="pp")
            nt.matmul(pp[:], wT[0:t_steps, 4 * b:4 * (b + 1)], hsb[:],
                      start=True, stop=True)
            so = stg_pool.tile([K, H], F32, tag="so")
            if b % 2 == 0:
                ns.copy(so[:], pp[:])
            else:
                nv.tensor_copy(so[:], pp[:])
            nc.sync.dma_start(d_out[K * b:K * (b + 1), :], so[:])

    nc.compile()
    return nc


def _host_prep(x, W_ih, W_hh, b_ih, b_hh, A1, A2, v1, lengths, label_len):
    assert int(label_len) == K
    # z gate columns: [f, i, g, o]
    perm = np.concatenate([np.arange(512, 1024), np.arange(0, 512),
                           np.arange(1024, 1536), np.arange(1536, 2048)])
    wih = np.ascontiguousarray(W_ih[perm].T, dtype=np.float32)          # [256, 2048]
    whh = np.ascontiguousarray(W_hh[perm].T, dtype=np.float32)          # [512, 2048]
    whh4 = whh.reshape(4, 128, G)
    whh_dr = [np.ascontiguousarray(
        np.concatenate([whh4[2 * j], whh4[2 * j + 1]], axis=1)
    ).astype(ml_dtypes.float8_e4m3) for j in range(2)]                  # [128, 2G] fp8
    bias = ((b_ih + b_hh)[perm]).astype(np.float32)                     # [2048]
    biasq = np.ascontiguousarray(bias.reshape(16, 128).T)               # [128, 16]
    u1 = (v1 @ A1)[0].astype(np.float32)                                # [256]
    u2 = (v1 @ A2)[0].astype(np.float32)                                # [512]
    u1c = np.ascontiguousarray(u1.reshape(2, 128).T)                    # [128, 2]
    u2c = np.ascontiguousarray(u2.reshape(4, 128).T * 16.0).astype(
        ml_dtypes.float8_e4m3)                                          # [128, 4] fp8

    i32s = np.zeros((128, 32), dtype=np.float32)
    i32s[np.arange(128), np.arange(128) % 32] = 1.0
    i128 = np.eye(128, dtype=np.float32)
    i128b = i128.astype(ml_dtypes.bfloat16)

    shared = dict(wih=wih, whh_dr0=whh_dr[0], whh_dr1=whh_dr[1], biasq=biasq,
                  u1c=u1c, u2c=u2c, i32s=i32s, i128=i128, i128b=i128b)

    in_maps = []
    for cidx in range(NC):
        sl = slice(cidx * BL, (cidx + 1) * BL)
        xc = x[:, sl, :]                                                # [T, 32, D]
        xT = np.ascontiguousarray(xc.reshape(T * BL, D).T, dtype=np.float32)
        ln = lengths[sl].astype(np.int64)
        t_start = np.maximum(ln - K, 0)
        t_k = t_start[:, None] + np.arange(K)[None, :]                  # [32, 4]
        valid = (t_k <= (ln[:, None] - 1))                              # [32, 4]
        tt = np.arange(T)
        mask = (tt[None, None, :] <= t_k[:, :, None]) & valid[:, :, None]  # [b, k, t]
        maskneg = np.where(mask, 0.0, NEG_INF).astype(np.float32)
        maskneg = np.ascontiguousarray(maskneg.reshape(BL, K * T))      # k-major cols
        in_maps.append(dict(shared, xT=xT, maskneg=maskneg,
                            valid=valid.astype(np.float32)))
    return in_maps


def kernel(**inputs) -> np.ndarray:
    inputs = {k: np.asarray(v) if not np.isscalar(v) else v for k, v in inputs.items()}
    in_maps = _host_prep(**inputs)
    if "nc" not in _cached:
        _cached["nc"] = _build_program()
    nc = _cached["nc"]
    res = run_bass_kernel_spmd(nc, in_maps, core_ids=list(range(NC)))
    outs = []
    for cidx in range(NC):
        o = res.results[cidx]["out"]                                    # [128, 512]
        outs.append(o.reshape(BL, K, H))
    return np.concatenate(outs, axis=0).astype(np.float32)              # [256, 4, 512]
